# revision 35
# baseline (speedup 1.0000x reference)
"""Trainium2 Bass kernel for MessagePassingLayerEC (gnn_message_passing).

Math (reference):
    src_proj  = node_values @ W_src + b_src            # [V, D]
    dest_proj = node_values @ W_dest + b_dest          # [V, D]
    msgs = relu(src_proj[edge_src] + dest_proj[edge_dest] + edge_emb[edge_cls])
    out  = segment_sum(msgs, edge_dest, V)             # [V, D]

Strategy (8 cores, edge-parallel, dest-contiguous ownership => no all-reduce):
  - Host sorts edges by dest; segments (dests) pack into groups of <= 96
    segments and <= 8 gather tiles (128 edges each).  Edges within a group
    sort by src and split into two windows (src < 25000 / >= 25000) so
    int16 gather indices are offsets into a 32k-row table slice — no
    half-table descriptor doubling.
  - Per super-batch of 8 groups, all w0 tiles then all w1 tiles; each
    window's slots gather with 2 dma_gathers => 4 gathers on 4 SWDGE
    queues (measured ~4x faster than single-queue).
  - dest_proj + edge_emb apply via ONE one-hot matmul per 128-edge block:
    combo rows 0..95 = the group's dest rows (resident SBUF slab,
    group-padded to 128 rows/chunk), rows 96..127 = 32 emb classes with
    biases folded (injected into the slab on device).  No dest gather.
  - Blocks stream in slot order with 8 concurrent PSUM segment
    accumulators (one per group of the super-batch); relu on ACT;
    per-super-batch indirect scatter-add into group-padded output rows.
"""

import sys

if "/opt/trn_rl_repo" not in sys.path:
    sys.path.insert(0, "/opt/trn_rl_repo")

import numpy as np
import ml_dtypes

BF16 = ml_dtypes.bfloat16

P = 128
NTILE_G = 8         # gather tiles (128-edge blocks) per group
GSLOT = NTILE_G * P          # edge slots per group (1024)
MAXSEG = 96         # segments per group (combo rows 96..127 = emb)
SB_G = 8            # groups per super-batch
SBT = SB_G * NTILE_G         # tiles per super-batch (64)
NQ = 4              # SWDGE queues
WSPLIT = 25000      # src window boundary
NC_CORES = 8

V_GLOBAL = 50000
E_GLOBAL = 640000
DIM = 128
NCLS = 32


def _round_up(x, m):
    return (x + m - 1) // m * m


def _wrap_idx16(flat):
    """dma_gather index layout: idx j -> [j%16, j//16], replicated 8x down
    partitions; packed into int32 pairs for PJRT friendliness."""
    n = flat.shape[0]
    assert n % 32 == 0
    w = np.zeros((P, n // 16), dtype=np.int16)
    blk = flat.reshape(n // 16, 16).T
    for g in range(8):
        w[g * 16:(g + 1) * 16, :] = blk
    return np.ascontiguousarray(w).view(np.int32)


# ---------------------------------------------------------------------------
# Host-side packing
# ---------------------------------------------------------------------------

def _host_pack(node_values, edge_src, edge_dest, edge_cls,
               W_src, b_src, W_dest, b_dest, edge_emb, n_cores=NC_CORES):
    V, D = node_values.shape
    E = edge_src.shape[0]

    order = np.argsort(edge_dest, kind="stable")
    ds_ = edge_dest[order].astype(np.int64)
    ss_ = edge_src[order].astype(np.int64)
    cs_ = edge_cls[order].astype(np.int64)

    first = np.empty(E, dtype=bool)
    first[0] = True
    first[1:] = ds_[1:] != ds_[:-1]
    seg_starts = np.flatnonzero(first)
    nseg = len(seg_starts)
    seg_ends = np.append(seg_starts[1:], E)
    seg_dest = ds_[seg_starts]
    seg_w1 = []  # per segment: count of src >= WSPLIT

    # split segments into n_cores chunks with ~equal edge counts
    seg_cut = [0]
    for k in range(1, n_cores):
        tgt = k * E // n_cores
        i = np.searchsorted(seg_starts, tgt)
        i = min(max(i, 1), nseg - 1)
        seg_cut.append(i)
    seg_cut.append(nseg)

    hi_counts = np.add.reduceat((ss_ >= WSPLIT).astype(np.int64), seg_starts)
    seg_lens = seg_ends - seg_starts

    # first-fit-decreasing bin packing per core: <=MAXSEG segs per group,
    # <=WCAP edges per src window.  Groups need not be dest-contiguous —
    # scat / slab maps are arbitrary.
    WCAP = NTILE_G // 2 * P
    core_groups = []
    for k in range(n_cores):
        lo, hi = seg_cut[k], seg_cut[k + 1]
        e1s = hi_counts[lo:hi]
        e0s = seg_lens[lo:hi] - e1s
        order_g = np.argsort(-(e0s + e1s), kind="stable")
        groups = []          # list of [seg indices]
        g_n0, g_n1 = [], []
        for oi in order_g:
            e0, e1 = int(e0s[oi]), int(e1s[oi])
            for gi in range(len(groups)):
                if (g_n0[gi] + e0 <= WCAP and g_n1[gi] + e1 <= WCAP
                        and len(groups[gi]) < MAXSEG):
                    groups[gi].append(lo + int(oi))
                    g_n0[gi] += e0
                    g_n1[gi] += e1
                    break
            else:
                groups.append([lo + int(oi)])
                g_n0.append(e0)
                g_n1.append(e1)
        core_groups.append(groups)

    NG = _round_up(max(len(g) for g in core_groups), SB_G)
    NSB = NG // SB_G
    SLAB_COLS = _round_up(NG * P, 2048)
    SCRATCH = NG * P
    OUT_ROWS = NG * P + 512
    VP = _round_up(V, 2048)

    nodesT = np.zeros((D, VP), dtype=BF16)
    nodesT[:, :V] = np.ascontiguousarray(node_values.T).astype(BF16)

    def _perm_cols(tbl):
        # column (c*512 + j*128 + p) <- node (c*512 + 4p + j): makes each
        # phase-1 output partition hold 4 consecutive rows (1KB descriptors)
        n = tbl.shape[1]
        pos = np.arange(n)
        node = (pos // 512) * 512 + 4 * (pos % 128) + (pos // 128) % 4
        return np.ascontiguousarray(tbl[:, node])

    emb_eff = (edge_emb + b_src[None, :] + b_dest[None, :]).astype(np.float32)
    emb_pad = np.zeros((P, D), dtype=BF16)
    emb_pad[MAXSEG:MAXSEG + NCLS, :] = emb_eff.astype(BF16)

    iota_t = np.tile(np.arange(P, dtype=np.float32), (P, 1)).astype(BF16)
    ident = np.eye(P, dtype=BF16)

    nodesT_perm = _perm_cols(nodesT)

    NBLK = NG * NTILE_G          # 128-edge blocks per core
    SBW = SBT * P // 32          # idx int32 cols per sb (256)
    SGW = SBT                    # sgid cols per sb (64)
    SCW = SB_G * P // 32         # scat int32 cols per sb (32)
    MW = SBW + SGW + SCW

    in_maps = []
    asm = []
    for k in range(n_cores):
        groups = core_groups[k]

        idx_flat = np.zeros(NG * GSLOT, dtype=np.int16)
        sgid = np.full((P, NBLK), 127.0, dtype=np.float32)
        oht = np.zeros((P, NBLK * P), dtype=BF16)
        scat = np.full((P, NG), SCRATCH, dtype=np.int32)
        slab_nodes = np.zeros(SLAB_COLS, dtype=np.int64)
        out_rows_l = []
        out_dest_l = []

        for gi, seglist in enumerate(groups):
            nsg = len(seglist)
            assert nsg <= MAXSEG
            scat[0:nsg, gi] = gi * P + np.arange(nsg)
            slab_nodes[gi * P:gi * P + nsg] = seg_dest[seglist]
            out_rows_l.append(gi * P + np.arange(nsg))
            out_dest_l.append(seg_dest[seglist])

            e_idx = np.concatenate([
                np.arange(int(seg_starts[s]), int(seg_ends[s]))
                for s in seglist])
            e_seg = np.concatenate([
                np.full(int(seg_ends[s] - seg_starts[s]), sl)
                for sl, s in enumerate(seglist)])
            src = ss_[e_idx]
            o = np.argsort(src, kind="stable")
            e_idx, e_seg, src = e_idx[o], e_seg[o], src[o]
            sb, g = gi // SB_G, gi % SB_G
            for w in range(2):
                m = (src >= WSPLIT) == (w == 1)
                ei, es, sr = e_idx[m], e_seg[m], src[m]
                n = len(ei)
                assert n <= WCAP, (gi, w, n)
                # w0 tiles of group g at sb-blocks [g*4, ..); w1 at 32 +
                i = np.arange(n)
                blk = (sb * SBT + w * SBT // 2 + g * (NTILE_G // 2)
                       + i // P)
                pp = i % P
                idx_flat[blk * P + pp] = sr - WSPLIT * w
                sgid[pp, blk] = es
                oht[es, blk * P + pp] = 1.0
                oht[MAXSEG + cs_[ei], blk * P + pp] = 1.0

        nodesT_slab = nodesT[:, slab_nodes]

        meta = np.zeros((P, NSB * MW), dtype=np.int32)
        for sb in range(NSB):
            c0 = sb * MW
            meta[:, c0:c0 + SBW] = _wrap_idx16(
                idx_flat[sb * SBT * P:(sb + 1) * SBT * P])
            meta[:, c0 + SBW:c0 + SBW + SGW] = \
                sgid[:, sb * SBT:(sb + 1) * SBT].view(np.int32)
            sflat = scat[:, sb * SB_G:(sb + 1) * SB_G].T.ravel().astype(np.int16)
            meta[:, c0 + SBW + SGW:c0 + MW] = _wrap_idx16(sflat)

        in_maps.append({
            "nodesT": nodesT_perm,
            "nodesT_slab": np.ascontiguousarray(nodesT_slab),
            "W_src": np.ascontiguousarray(W_src).astype(BF16),
            "W_dest": np.ascontiguousarray(W_dest).astype(BF16),
            "emb_pad": emb_pad,
            "iota_t": iota_t,
            "ident": ident,
            "meta": meta,
            "onehotT": oht,
        })
        asm.append((np.concatenate(out_rows_l), np.concatenate(out_dest_l)))

    params = dict(NG=int(NG), SLAB_COLS=int(SLAB_COLS),
                  OUT_ROWS=int(OUT_ROWS), VP=int(VP), D=int(D))
    return in_maps, asm, params


# ---------------------------------------------------------------------------
# Bass kernel
# ---------------------------------------------------------------------------

def build_kernel(params):
    import concourse.bass as bass
    import concourse.mybir as mybir
    import concourse.tile as tile
    from concourse import bacc

    NG = params["NG"]
    SLAB_COLS = params["SLAB_COLS"]
    OUT_ROWS = params["OUT_ROWS"]
    VP = params["VP"]
    D = params["D"]
    NSB = NG // SB_G
    NBLK = NG * NTILE_G
    SBW = SBT * P // 32
    SGW = SBT
    SCW = SB_G * P // 32
    MW = SBW + SGW + SCW

    f32 = mybir.dt.float32
    bf16 = mybir.dt.bfloat16
    i32 = mybir.dt.int32
    i16 = mybir.dt.int16

    nc = bacc.Bacc("TRN2", target_bir_lowering=False, num_swdge_queues=NQ)

    nodesT = nc.dram_tensor("nodesT", [D, VP], bf16, kind="ExternalInput")
    nodesT_slab = nc.dram_tensor("nodesT_slab", [D, SLAB_COLS], bf16,
                                 kind="ExternalInput")
    W_src = nc.dram_tensor("W_src", [D, D], bf16, kind="ExternalInput")
    W_dest = nc.dram_tensor("W_dest", [D, D], bf16, kind="ExternalInput")
    emb_pad = nc.dram_tensor("emb_pad", [P, D], bf16, kind="ExternalInput")
    iota_t = nc.dram_tensor("iota_t", [P, P], bf16, kind="ExternalInput")
    ident_d = nc.dram_tensor("ident", [P, P], bf16, kind="ExternalInput")
    meta = nc.dram_tensor("meta", [P, NSB * MW], i32, kind="ExternalInput")
    onehotT = nc.dram_tensor("onehotT", [P, NBLK * P], bf16,
                             kind="ExternalInput")

    src_tbl = nc.dram_tensor("src_tbl", [VP, D], bf16, kind="Internal")
    dest_proj = nc.dram_tensor("dest_proj", [P, SLAB_COLS], bf16,
                               kind="Internal")
    out = nc.dram_tensor("out", [OUT_ROWS, D], f32, kind="ExternalOutput")

    with tile.TileContext(nc) as tc, tc.tile_pool(name="const", bufs=1) as cpool:
        w_src_sb = cpool.tile([D, D], bf16, tag="wsrc")
        nc.sync.dma_start(w_src_sb[:], W_src[:, :])
        w_dest_sb = cpool.tile([D, D], bf16, tag="wdest")
        nc.sync.dma_start(w_dest_sb[:], W_dest[:, :])
        emb_sb = cpool.tile([P, D], bf16, tag="embp")
        nc.sync.dma_start(emb_sb[:], emb_pad[:, :])
        iota_sb = cpool.tile([P, P], bf16, tag="iota")
        nc.sync.dma_start(iota_sb[:], iota_t[:, :])
        ident_sb = cpool.tile([P, P], bf16, tag="ident")
        nc.sync.dma_start(ident_sb[:], ident_d[:, :])
        slab_sb = cpool.tile([P, NG, D], bf16, tag="slab")

        # ---------------- phase 1: projections (bf16) ----------------
        with (
            tc.tile_pool(name="p1", bufs=3) as p1pool,
            tc.tile_pool(name="p1ps", bufs=2, space="PSUM") as p1ps,
        ):
            def proj_pass(n_cols, src_dram, w_sb, dview, permuted):
                nsup = n_cols // 2048
                for su in range(nsup):
                    nt_sb = p1pool.tile([D, 2048], bf16, tag="p1in")
                    nc.sync.dma_start(
                        nt_sb[:], src_dram[:, su * 2048:(su + 1) * 2048])
                    ob = p1pool.tile([P, 4, 512], bf16, tag="p1out")
                    for cc in range(4):
                        ps = p1ps.tile([P, 512], f32, tag="p1ps")
                        for j in range(4):
                            nc.tensor.matmul(
                                ps[:, j * P:(j + 1) * P],
                                lhsT=nt_sb[:, cc * 512 + j * P:
                                           cc * 512 + (j + 1) * P],
                                rhs=w_sb[:],
                                start=True, stop=True,
                            )
                        nc.scalar.activation(
                            ob[:, cc, :], ps[:],
                            mybir.ActivationFunctionType.Copy)
                    if permuted:
                        nc.sync.dma_start(
                            dview[:, su * 4:(su + 1) * 4, :], ob[:])
                    else:
                        nc.sync.dma_start(
                            dview[:, su * 2048:(su + 1) * 2048],
                            ob[:].rearrange("p c e -> p (c e)"))

            proj_pass(VP, nodesT, w_src_sb,
                      src_tbl[:, :].rearrange("(c p r) d -> p c (r d)",
                                              p=P, r=4), True)
            proj_pass(SLAB_COLS, nodesT_slab, w_dest_sb,
                      dest_proj[:, :], False)

        # load resident slab, inject emb rows at partitions 96..127
        nc.sync.dma_start(
            slab_sb[:], dest_proj[:, 0:NG * P].rearrange(
                "p (g d) -> p g d", g=NG))
        for g in range(NG):
            nc.any.tensor_copy(slab_sb[MAXSEG:P, g, :],
                               emb_sb[MAXSEG:P, :])

        # zero the output slab (scatter-add target; degree-0 rows stay 0)
        with tc.tile_pool(name="zz", bufs=1) as zpool:
            zt = zpool.tile([P, 512], f32, tag="zt")
            nc.vector.memset(zt[:], 0.0)
            zview = out[:, :].rearrange("(c p r) d -> p c (r d)", p=P, r=4)
            for zc in range(OUT_ROWS // 512):
                nc.sync.dma_start(zview[:, zc:zc + 1, :],
                                  zt[:].rearrange("p (o f) -> p o f", o=1))

        # ---------------- phase 2: edges ----------------
        with (
            tc.tile_pool(name="meta", bufs=3) as mpool,
            tc.tile_pool(name="oht", bufs=3) as opool,
            tc.tile_pool(name="gath", bufs=3) as gpool,
            tc.tile_pool(name="work", bufs=4) as wpool,
            tc.tile_pool(name="msgs", bufs=3) as mspool,
            tc.tile_pool(name="segout", bufs=3) as spool,
            tc.tile_pool(name="psmsg", bufs=3, space="PSUM") as psmsg,
            tc.tile_pool(name="psseg", bufs=2, space="PSUM") as psseg,
        ):
            for sb in range(NSB):
                mt = mpool.tile([P, MW], i32, tag="mt")
                nc.sync.dma_start(mt[:], meta[:, sb * MW:(sb + 1) * MW])
                ia = mt[:, 0:SBW]
                sgid = mt[:, SBW:SBW + SGW].bitcast(f32)
                sc16 = mt[:, SBW + SGW:MW]
                oht = opool.tile([P, SBT * P], bf16, tag="oht")
                nc.sync.dma_start(
                    oht[:], onehotT[:, sb * SBT * P:(sb + 1) * SBT * P])

                ga = gpool.tile([P, SBT, D], bf16, tag="ga")
                # 4 sub-gathers: windows (w0 tiles | w1 tiles), each halved
                HT = SBT // 2
                for q in range(NQ):
                    t0c = q * (SBT // NQ)
                    t1c = t0c + SBT // NQ
                    base = 0 if q < 2 else WSPLIT
                    nidx = (t1c - t0c) * P
                    nc.gpsimd.dma_gather(
                        ga[:, t0c:t1c, :],
                        src_tbl[base:min(base + 32768, VP), :],
                        ia[:, t0c * 4:t1c * 4].bitcast(i16),
                        nidx, nidx, D,
                        single_packet=False, queue_num=q)

                # chunk c covers blocks [4c, 4c+4) -> all of group c % 8
                # (w0 chunks 0..7, w1 chunks 8..15); process each group's
                # two chunks back-to-back so only one PSUM segment
                # accumulation window is open at a time.
                seg_sb = spool.tile([P, SB_G, D], f32, tag="segsb")
                for gl in range(SB_G):
                    ps_seg = psseg.tile([P, P], f32, tag="psseg")
                    for c in (gl, gl + SB_G):
                        ps_m = psmsg.tile([P, 512], f32, tag="psmsg")
                        for j in range(4):
                            blk = c * 4 + j
                            nc.tensor.matmul(
                                ps_m[:, j * P:(j + 1) * P],
                                lhsT=oht[:, blk * P:(blk + 1) * P],
                                rhs=slab_sb[:, sb * SB_G + gl, :],
                                start=True, stop=True,
                            )
                        t3 = wpool.tile([P, 512], f32, tag="t3")
                        nc.vector.tensor_tensor(
                            out=t3[:],
                            in0=ga[:, c * 4:(c + 1) * 4, :].rearrange(
                                "p t e -> p (t e)"),
                            in1=ps_m[:],
                            op=mybir.AluOpType.add)
                        msgs = mspool.tile([P, 512], bf16, tag="msgs")
                        nc.scalar.activation(
                            msgs[:], t3[:],
                            mybir.ActivationFunctionType.Relu)
                        for j in range(4):
                            blk = c * 4 + j
                            gt = wpool.tile([P, P], bf16, tag="gt")
                            nc.any.tensor_scalar(
                                out=gt[:], in0=iota_sb[:],
                                scalar1=sgid[:, blk:blk + 1], scalar2=None,
                                op0=mybir.AluOpType.is_equal)
                            nc.tensor.matmul(
                                ps_seg[:],
                                lhsT=gt[:], rhs=msgs[:, j * P:(j + 1) * P],
                                start=(c == gl and j == 0),
                                stop=(c == gl + SB_G and j == 3))
                    nc.any.tensor_copy(seg_sb[:, gl, :], ps_seg[:])
                nc.gpsimd.dma_scatter_add(
                    out[:, :], seg_sb[:], sc16.bitcast(i16),
                    SB_G * P, SB_G * P, D, single_packet=False,
                    queue_num=sb % NQ)

    nc.compile()
    return nc


# ---------------------------------------------------------------------------
# Entry point
# ---------------------------------------------------------------------------

def kernel(**inputs):
    node_values = np.asarray(inputs["node_values"], dtype=np.float32)
    edge_src = np.asarray(inputs["edge_src"], dtype=np.int32)
    edge_dest = np.asarray(inputs["edge_dest"], dtype=np.int32)
    edge_cls = np.asarray(inputs["edge_cls"], dtype=np.int32)
    W_src = np.asarray(inputs["W_src"], dtype=np.float32)
    b_src = np.asarray(inputs["b_src"], dtype=np.float32)
    W_dest = np.asarray(inputs["W_dest"], dtype=np.float32)
    b_dest = np.asarray(inputs["b_dest"], dtype=np.float32)
    edge_emb = np.asarray(inputs["edge_emb"], dtype=np.float32)

    V = node_values.shape[0]

    in_maps, asm, params = _host_pack(
        node_values, edge_src, edge_dest, edge_cls,
        W_src, b_src, W_dest, b_dest, edge_emb)

    nc = build_kernel(params)

    from concourse.bass_utils import run_bass_kernel_spmd
    res = run_bass_kernel_spmd(nc, in_maps, core_ids=list(range(NC_CORES)))

    out = np.zeros((V, DIM), dtype=np.float32)
    for k in range(NC_CORES):
        rows, dests = asm[k]
        out[dests] = np.asarray(res.results[k]["out"])[rows]
    return out


if __name__ == "__main__":
    rng = np.random.default_rng(0)
    V, E = V_GLOBAL, E_GLOBAL
    ins = {
        "node_values": rng.normal(size=(V, DIM)).astype(np.float32),
        "edge_src": rng.integers(0, V, size=E).astype(np.int32),
        "edge_dest": rng.integers(0, V, size=E).astype(np.int32),
        "edge_cls": rng.integers(0, NCLS, size=E).astype(np.int32),
        "W_src": (rng.normal(size=(DIM, DIM)) / np.sqrt(DIM)).astype(np.float32),
        "b_src": np.zeros(DIM, dtype=np.float32),
        "W_dest": (rng.normal(size=(DIM, DIM)) / np.sqrt(DIM)).astype(np.float32),
        "b_dest": np.zeros(DIM, dtype=np.float32),
        "edge_emb": rng.normal(size=(NCLS, DIM)).astype(np.float32),
    }
    out = kernel(**ins)
    print("out", out.shape, out.dtype, float(np.abs(out).sum()))


# revision 37
# speedup vs baseline: 1.9916x; 1.9916x over previous
"""Trainium2 Bass kernel for MessagePassingLayerEC (gnn_message_passing).

Math (reference):
    src_proj  = node_values @ W_src + b_src            # [V, D]
    dest_proj = node_values @ W_dest + b_dest          # [V, D]
    msgs = relu(src_proj[edge_src] + dest_proj[edge_dest] + edge_emb[edge_cls])
    out  = segment_sum(msgs, edge_dest, V)             # [V, D]

Strategy (8 cores, edge-parallel, dest-contiguous ownership => no all-reduce):
  - Host sorts edges by dest; segments (dests) pack into groups of <= 96
    segments and <= 8 gather tiles (128 edges each).  Edges within a group
    sort by src and split into two windows (src < 25000 / >= 25000) so
    int16 gather indices are offsets into a 32k-row table slice — no
    half-table descriptor doubling.
  - Per super-batch of 8 groups, all w0 tiles then all w1 tiles; each
    window's slots gather with 2 dma_gathers => 4 gathers on 4 SWDGE
    queues (measured ~4x faster than single-queue).
  - dest_proj + edge_emb apply via ONE one-hot matmul per 128-edge block:
    combo rows 0..95 = the group's dest rows (resident SBUF slab,
    group-padded to 128 rows/chunk), rows 96..127 = 32 emb classes with
    biases folded (injected into the slab on device).  No dest gather.
  - Blocks stream in slot order with 8 concurrent PSUM segment
    accumulators (one per group of the super-batch); relu on ACT;
    per-super-batch indirect scatter-add into group-padded output rows.
"""

import sys

if "/opt/trn_rl_repo" not in sys.path:
    sys.path.insert(0, "/opt/trn_rl_repo")

import numpy as np
import ml_dtypes

BF16 = ml_dtypes.bfloat16

P = 128
NTILE_G = 8         # gather tiles (128-edge blocks) per group
GSLOT = NTILE_G * P          # edge slots per group (1024)
MAXSEG = 96         # segments per group (combo rows 96..127 = emb)
SB_G = 8            # groups per super-batch
SBT = SB_G * NTILE_G         # tiles per super-batch (64)
NQ = 4              # SWDGE queues
WSPLIT = 25000      # src window boundary
NC_CORES = 8

V_GLOBAL = 50000
E_GLOBAL = 640000
DIM = 128
NCLS = 32


def _round_up(x, m):
    return (x + m - 1) // m * m


def _wrap_idx16(flat):
    """dma_gather index layout: idx j -> [j%16, j//16], replicated 8x down
    partitions; packed into int32 pairs for PJRT friendliness."""
    n = flat.shape[0]
    assert n % 32 == 0
    w = np.zeros((P, n // 16), dtype=np.int16)
    blk = flat.reshape(n // 16, 16).T
    for g in range(8):
        w[g * 16:(g + 1) * 16, :] = blk
    return np.ascontiguousarray(w).view(np.int32)


# ---------------------------------------------------------------------------
# Host-side packing
# ---------------------------------------------------------------------------

def _host_pack(node_values, edge_src, edge_dest, edge_cls,
               W_src, b_src, W_dest, b_dest, edge_emb, n_cores=NC_CORES):
    V, D = node_values.shape
    E = edge_src.shape[0]

    order = np.argsort(edge_dest, kind="stable")
    ds_ = edge_dest[order].astype(np.int64)
    ss_ = edge_src[order].astype(np.int64)
    cs_ = edge_cls[order].astype(np.int64)

    first = np.empty(E, dtype=bool)
    first[0] = True
    first[1:] = ds_[1:] != ds_[:-1]
    seg_starts = np.flatnonzero(first)
    nseg = len(seg_starts)
    seg_ends = np.append(seg_starts[1:], E)
    seg_dest = ds_[seg_starts]
    seg_w1 = []  # per segment: count of src >= WSPLIT

    # split segments into n_cores chunks with ~equal edge counts
    seg_cut = [0]
    for k in range(1, n_cores):
        tgt = k * E // n_cores
        i = np.searchsorted(seg_starts, tgt)
        i = min(max(i, 1), nseg - 1)
        seg_cut.append(i)
    seg_cut.append(nseg)

    hi_counts = np.add.reduceat((ss_ >= WSPLIT).astype(np.int64), seg_starts)
    seg_lens = seg_ends - seg_starts

    # greedy group packing per core: <=MAXSEG segs, <=WCAP edges per window
    WCAP = NTILE_G // 2 * P
    core_groups = []
    for k in range(n_cores):
        lo, hi = seg_cut[k], seg_cut[k + 1]
        groups = []
        g_lo = lo
        n0 = n1 = gseg = 0
        for g in range(lo, hi):
            e1 = int(hi_counts[g])
            e0 = int(seg_lens[g]) - e1
            if g > g_lo and (n0 + e0 > WCAP or n1 + e1 > WCAP
                             or gseg + 1 > MAXSEG):
                groups.append((g_lo, g))
                g_lo = g
                n0 = n1 = gseg = 0
            n0 += e0
            n1 += e1
            gseg += 1
        groups.append((g_lo, hi))
        core_groups.append(groups)

    NG = _round_up(max(len(g) for g in core_groups), SB_G)
    NSB = NG // SB_G
    SLAB_COLS = _round_up(NG * P, 2048)
    SCRATCH = NG * P
    OUT_ROWS = NG * P + 512
    VP = _round_up(V, 2048)

    nodesT = np.zeros((D, VP), dtype=BF16)
    nodesT[:, :V] = np.ascontiguousarray(node_values.T).astype(BF16)

    def _perm_cols(tbl):
        # column (c*512 + j*128 + p) <- node (c*512 + 4p + j): makes each
        # phase-1 output partition hold 4 consecutive rows (1KB descriptors)
        n = tbl.shape[1]
        pos = np.arange(n)
        node = (pos // 512) * 512 + 4 * (pos % 128) + (pos // 128) % 4
        return np.ascontiguousarray(tbl[:, node])

    emb_eff = (edge_emb + b_src[None, :] + b_dest[None, :]).astype(np.float32)
    emb_pad = np.zeros((P, D), dtype=BF16)
    emb_pad[MAXSEG:MAXSEG + NCLS, :] = emb_eff.astype(BF16)

    iota_t = np.tile(np.arange(P, dtype=np.float32), (P, 1)).astype(BF16)
    ident = np.eye(P, dtype=BF16)

    nodesT_perm = _perm_cols(nodesT)

    NBLK = NG * NTILE_G          # 128-edge blocks per core
    SBW = SBT * P // 32          # idx int32 cols per sb (256)
    SGW = SBT                    # sgid cols per sb (64)
    SCW = SB_G * P // 32         # scat int32 cols per sb (32)
    MW = SBW + SGW + SCW

    in_maps = []
    asm = []
    for k in range(n_cores):
        groups = core_groups[k]

        idx_flat = np.zeros(NG * GSLOT, dtype=np.int16)
        sgid = np.full((P, NBLK), 127.0, dtype=np.float32)
        oht = np.zeros((P, NBLK * P), dtype=BF16)
        scat = np.full((P, NG), SCRATCH, dtype=np.int32)
        slab_nodes = np.zeros(SLAB_COLS, dtype=np.int64)
        out_rows_l = []
        out_dest_l = []

        for gi, (glo, ghi) in enumerate(groups):
            nsg = ghi - glo
            assert nsg <= MAXSEG
            scat[0:nsg, gi] = gi * P + np.arange(nsg)
            slab_nodes[gi * P:gi * P + nsg] = seg_dest[glo:ghi]
            out_rows_l.append(gi * P + np.arange(nsg))
            out_dest_l.append(seg_dest[glo:ghi])

            e_idx = np.concatenate([
                np.arange(int(seg_starts[s]), int(seg_ends[s]))
                for s in range(glo, ghi)])
            e_seg = np.concatenate([
                np.full(int(seg_ends[s] - seg_starts[s]), s - glo)
                for s in range(glo, ghi)])
            src = ss_[e_idx]
            o = np.argsort(src, kind="stable")
            e_idx, e_seg, src = e_idx[o], e_seg[o], src[o]
            sb, g = gi // SB_G, gi % SB_G
            for w in range(2):
                m = (src >= WSPLIT) == (w == 1)
                ei, es, sr = e_idx[m], e_seg[m], src[m]
                n = len(ei)
                assert n <= WCAP, (gi, w, n)
                # w0 tiles of group g at sb-blocks [g*4, ..); w1 at 32 +
                i = np.arange(n)
                blk = (sb * SBT + w * SBT // 2 + g * (NTILE_G // 2)
                       + i // P)
                pp = i % P
                idx_flat[blk * P + pp] = sr - WSPLIT * w
                sgid[pp, blk] = es
                oht[es, blk * P + pp] = 1.0
                oht[MAXSEG + cs_[ei], blk * P + pp] = 1.0

        nodesT_slab = nodesT[:, slab_nodes]

        meta = np.zeros((P, NSB * MW), dtype=np.int32)
        for sb in range(NSB):
            c0 = sb * MW
            meta[:, c0:c0 + SBW] = _wrap_idx16(
                idx_flat[sb * SBT * P:(sb + 1) * SBT * P])
            meta[:, c0 + SBW:c0 + SBW + SGW] = \
                sgid[:, sb * SBT:(sb + 1) * SBT].view(np.int32)
            sflat = scat[:, sb * SB_G:(sb + 1) * SB_G].T.ravel().astype(np.int16)
            meta[:, c0 + SBW + SGW:c0 + MW] = _wrap_idx16(sflat)

        in_maps.append({
            "nodesT": nodesT_perm,
            "nodesT_slab": np.ascontiguousarray(nodesT_slab),
            "W_src": np.ascontiguousarray(W_src).astype(BF16),
            "W_dest": np.ascontiguousarray(W_dest).astype(BF16),
            "emb_pad": emb_pad,
            "iota_t": iota_t,
            "ident": ident,
            "meta": meta,
            "onehotT": oht,
        })
        asm.append((np.concatenate(out_rows_l), np.concatenate(out_dest_l)))

    params = dict(NG=int(NG), SLAB_COLS=int(SLAB_COLS),
                  OUT_ROWS=int(OUT_ROWS), VP=int(VP), D=int(D))
    return in_maps, asm, params


# ---------------------------------------------------------------------------
# Bass kernel
# ---------------------------------------------------------------------------

def build_kernel(params):
    import concourse.bass as bass
    import concourse.mybir as mybir
    import concourse.tile as tile
    from concourse import bacc

    NG = params["NG"]
    SLAB_COLS = params["SLAB_COLS"]
    OUT_ROWS = params["OUT_ROWS"]
    VP = params["VP"]
    D = params["D"]
    NSB = NG // SB_G
    NBLK = NG * NTILE_G
    SBW = SBT * P // 32
    SGW = SBT
    SCW = SB_G * P // 32
    MW = SBW + SGW + SCW

    f32 = mybir.dt.float32
    bf16 = mybir.dt.bfloat16
    i32 = mybir.dt.int32
    i16 = mybir.dt.int16

    nc = bacc.Bacc("TRN2", target_bir_lowering=False, num_swdge_queues=NQ)

    nodesT = nc.dram_tensor("nodesT", [D, VP], bf16, kind="ExternalInput")
    nodesT_slab = nc.dram_tensor("nodesT_slab", [D, SLAB_COLS], bf16,
                                 kind="ExternalInput")
    W_src = nc.dram_tensor("W_src", [D, D], bf16, kind="ExternalInput")
    W_dest = nc.dram_tensor("W_dest", [D, D], bf16, kind="ExternalInput")
    emb_pad = nc.dram_tensor("emb_pad", [P, D], bf16, kind="ExternalInput")
    iota_t = nc.dram_tensor("iota_t", [P, P], bf16, kind="ExternalInput")
    ident_d = nc.dram_tensor("ident", [P, P], bf16, kind="ExternalInput")
    meta = nc.dram_tensor("meta", [P, NSB * MW], i32, kind="ExternalInput")
    onehotT = nc.dram_tensor("onehotT", [P, NBLK * P], bf16,
                             kind="ExternalInput")

    src_tbl = nc.dram_tensor("src_tbl", [VP, D], bf16, kind="Internal")
    dest_proj = nc.dram_tensor("dest_proj", [P, SLAB_COLS], bf16,
                               kind="Internal")
    out = nc.dram_tensor("out", [OUT_ROWS, D], f32, kind="ExternalOutput")

    with tile.TileContext(nc) as tc, tc.tile_pool(name="const", bufs=1) as cpool:
        w_src_sb = cpool.tile([D, D], bf16, tag="wsrc")
        nc.sync.dma_start(w_src_sb[:], W_src[:, :])
        w_dest_sb = cpool.tile([D, D], bf16, tag="wdest")
        nc.sync.dma_start(w_dest_sb[:], W_dest[:, :])
        emb_sb = cpool.tile([P, D], bf16, tag="embp")
        nc.sync.dma_start(emb_sb[:], emb_pad[:, :])
        iota_sb = cpool.tile([P, P], bf16, tag="iota")
        nc.sync.dma_start(iota_sb[:], iota_t[:, :])
        ident_sb = cpool.tile([P, P], bf16, tag="ident")
        nc.sync.dma_start(ident_sb[:], ident_d[:, :])
        slab_sb = cpool.tile([P, NG, D], bf16, tag="slab")

        # ---------------- phase 1: projections (bf16) ----------------
        with (
            tc.tile_pool(name="p1", bufs=3) as p1pool,
            tc.tile_pool(name="p1ps", bufs=2, space="PSUM") as p1ps,
        ):
            def proj_pass(n_cols, src_dram, w_sb, dview, permuted):
                nsup = n_cols // 2048
                for su in range(nsup):
                    nt_sb = p1pool.tile([D, 2048], bf16, tag="p1in")
                    nc.sync.dma_start(
                        nt_sb[:], src_dram[:, su * 2048:(su + 1) * 2048])
                    ob = p1pool.tile([P, 4, 512], bf16, tag="p1out")
                    for cc in range(4):
                        ps = p1ps.tile([P, 512], f32, tag="p1ps")
                        for j in range(4):
                            nc.tensor.matmul(
                                ps[:, j * P:(j + 1) * P],
                                lhsT=nt_sb[:, cc * 512 + j * P:
                                           cc * 512 + (j + 1) * P],
                                rhs=w_sb[:],
                                start=True, stop=True,
                            )
                        nc.scalar.activation(
                            ob[:, cc, :], ps[:],
                            mybir.ActivationFunctionType.Copy)
                    if permuted:
                        nc.sync.dma_start(
                            dview[:, su * 4:(su + 1) * 4, :], ob[:])
                    else:
                        nc.sync.dma_start(
                            dview[:, su * 2048:(su + 1) * 2048],
                            ob[:].rearrange("p c e -> p (c e)"))

            proj_pass(VP, nodesT, w_src_sb,
                      src_tbl[:, :].rearrange("(c p r) d -> p c (r d)",
                                              p=P, r=4), True)
            proj_pass(SLAB_COLS, nodesT_slab, w_dest_sb,
                      dest_proj[:, :], False)

        # load resident slab, inject emb rows at partitions 96..127
        nc.sync.dma_start(
            slab_sb[:], dest_proj[:, 0:NG * P].rearrange(
                "p (g d) -> p g d", g=NG))
        for g in range(NG):
            nc.any.tensor_copy(slab_sb[MAXSEG:P, g, :],
                               emb_sb[MAXSEG:P, :])

        # zero the output slab (scatter-add target; degree-0 rows stay 0)
        with tc.tile_pool(name="zz", bufs=1) as zpool:
            zt = zpool.tile([P, 512], f32, tag="zt")
            nc.vector.memset(zt[:], 0.0)
            zview = out[:, :].rearrange("(c p r) d -> p c (r d)", p=P, r=4)
            for zc in range(OUT_ROWS // 512):
                nc.sync.dma_start(zview[:, zc:zc + 1, :],
                                  zt[:].rearrange("p (o f) -> p o f", o=1))

        # ---------------- phase 2: edges ----------------
        with (
            tc.tile_pool(name="meta", bufs=4) as mpool,
            tc.tile_pool(name="oht", bufs=4) as opool,
            tc.tile_pool(name="gath", bufs=4) as gpool,
            tc.tile_pool(name="work", bufs=4) as wpool,
            tc.tile_pool(name="msgs", bufs=3) as mspool,
            tc.tile_pool(name="segout", bufs=3) as spool,
            tc.tile_pool(name="psmsg", bufs=3, space="PSUM") as psmsg,
            tc.tile_pool(name="psseg", bufs=2, space="PSUM") as psseg,
        ):
            for sb in range(NSB):
                mt = mpool.tile([P, MW], i32, tag="mt")
                nc.sync.dma_start(mt[:], meta[:, sb * MW:(sb + 1) * MW])
                ia = mt[:, 0:SBW]
                sgid = mt[:, SBW:SBW + SGW].bitcast(f32)
                sc16 = mt[:, SBW + SGW:MW]
                oht = opool.tile([P, SBT * P], bf16, tag="oht")
                nc.sync.dma_start(
                    oht[:], onehotT[:, sb * SBT * P:(sb + 1) * SBT * P])

                ga = gpool.tile([P, SBT, D], bf16, tag="ga")
                # 4 sub-gathers: windows (w0 tiles | w1 tiles), each halved
                HT = SBT // 2
                for q in range(NQ):
                    t0c = q * (SBT // NQ)
                    t1c = t0c + SBT // NQ
                    base = 0 if q < 2 else WSPLIT
                    nidx = (t1c - t0c) * P
                    nc.gpsimd.dma_gather(
                        ga[:, t0c:t1c, :],
                        src_tbl[base:min(base + 32768, VP), :],
                        ia[:, t0c * 4:t1c * 4].bitcast(i16),
                        nidx, nidx, D,
                        single_packet=False, queue_num=q)

                # chunk c covers blocks [4c, 4c+4) -> all of group c % 8
                # (w0 chunks 0..7, w1 chunks 8..15); process each group's
                # two chunks back-to-back so only one PSUM segment
                # accumulation window is open at a time.
                seg_sb = spool.tile([P, SB_G, D], f32, tag="segsb")
                for gl in range(SB_G):
                    ps_seg = psseg.tile([P, P], f32, tag="psseg")
                    for c in (gl, gl + SB_G):
                        ps_m = psmsg.tile([P, 512], f32, tag="psmsg")
                        for j in range(4):
                            blk = c * 4 + j
                            nc.tensor.matmul(
                                ps_m[:, j * P:(j + 1) * P],
                                lhsT=oht[:, blk * P:(blk + 1) * P],
                                rhs=slab_sb[:, sb * SB_G + gl, :],
                                start=True, stop=True,
                            )
                        t3 = wpool.tile([P, 512], f32, tag="t3")
                        nc.vector.tensor_tensor(
                            out=t3[:],
                            in0=ga[:, c * 4:(c + 1) * 4, :].rearrange(
                                "p t e -> p (t e)"),
                            in1=ps_m[:],
                            op=mybir.AluOpType.add)
                        msgs = mspool.tile([P, 512], bf16, tag="msgs")
                        nc.scalar.activation(
                            msgs[:], t3[:],
                            mybir.ActivationFunctionType.Relu)
                        for j in range(4):
                            blk = c * 4 + j
                            gt = wpool.tile([P, P], bf16, tag="gt")
                            nc.any.tensor_scalar(
                                out=gt[:], in0=iota_sb[:],
                                scalar1=sgid[:, blk:blk + 1], scalar2=None,
                                op0=mybir.AluOpType.is_equal)
                            nc.tensor.matmul(
                                ps_seg[:],
                                lhsT=gt[:], rhs=msgs[:, j * P:(j + 1) * P],
                                start=(c == gl and j == 0),
                                stop=(c == gl + SB_G and j == 3))
                    nc.any.tensor_copy(seg_sb[:, gl, :], ps_seg[:])
                nc.gpsimd.dma_scatter_add(
                    out[:, :], seg_sb[:], sc16.bitcast(i16),
                    SB_G * P, SB_G * P, D, single_packet=False,
                    queue_num=sb % NQ)

    nc.compile()
    return nc


# ---------------------------------------------------------------------------
# Entry point
# ---------------------------------------------------------------------------

def kernel(**inputs):
    node_values = np.asarray(inputs["node_values"], dtype=np.float32)
    edge_src = np.asarray(inputs["edge_src"], dtype=np.int32)
    edge_dest = np.asarray(inputs["edge_dest"], dtype=np.int32)
    edge_cls = np.asarray(inputs["edge_cls"], dtype=np.int32)
    W_src = np.asarray(inputs["W_src"], dtype=np.float32)
    b_src = np.asarray(inputs["b_src"], dtype=np.float32)
    W_dest = np.asarray(inputs["W_dest"], dtype=np.float32)
    b_dest = np.asarray(inputs["b_dest"], dtype=np.float32)
    edge_emb = np.asarray(inputs["edge_emb"], dtype=np.float32)

    V = node_values.shape[0]

    in_maps, asm, params = _host_pack(
        node_values, edge_src, edge_dest, edge_cls,
        W_src, b_src, W_dest, b_dest, edge_emb)

    nc = build_kernel(params)

    from concourse.bass_utils import run_bass_kernel_spmd
    res = run_bass_kernel_spmd(nc, in_maps, core_ids=list(range(NC_CORES)))

    out = np.zeros((V, DIM), dtype=np.float32)
    for k in range(NC_CORES):
        rows, dests = asm[k]
        out[dests] = np.asarray(res.results[k]["out"])[rows]
    return out


if __name__ == "__main__":
    rng = np.random.default_rng(0)
    V, E = V_GLOBAL, E_GLOBAL
    ins = {
        "node_values": rng.normal(size=(V, DIM)).astype(np.float32),
        "edge_src": rng.integers(0, V, size=E).astype(np.int32),
        "edge_dest": rng.integers(0, V, size=E).astype(np.int32),
        "edge_cls": rng.integers(0, NCLS, size=E).astype(np.int32),
        "W_src": (rng.normal(size=(DIM, DIM)) / np.sqrt(DIM)).astype(np.float32),
        "b_src": np.zeros(DIM, dtype=np.float32),
        "W_dest": (rng.normal(size=(DIM, DIM)) / np.sqrt(DIM)).astype(np.float32),
        "b_dest": np.zeros(DIM, dtype=np.float32),
        "edge_emb": rng.normal(size=(NCLS, DIM)).astype(np.float32),
    }
    out = kernel(**ins)
    print("out", out.shape, out.dtype, float(np.abs(out).sum()))
